# revision 30
# baseline (speedup 1.0000x reference)
"""GAT block (graph attention) Bass/Tile kernel for Trainium2, 8 NeuronCores.

Full-input contract: kernel(x=(8,2048,128), W=(128,64), a=(128,1)) -> (8,2048,64).
Sharding: data-parallel over batch - one batch element per core, W/a replicated,
zero inter-core communication; host transposes x per core and un-transposes the
per-core outputs.

Algorithm (per core, N=2048, Fin=128, F=64):
  e[i,j] = lrelu(s1_i + s2_j);  att = softmax(e, axis=0);  out = lrelu(att @ h).
  exp(lrelu(z)) is exactly separable on each side of the kink:
     z>0:  e^{s1_i} e^{s2_j}   (u_i v_j)
     z<=0: e^{.2 s1_i} e^{.2 s2_j}  (p_i q_j)
  so with threshold sums A(th)=sum_{s1_i>th} u_i, B(th)=sum p_i (and the
  mirrored C(th,f)=sum_{s2_j>th} v_j g_jf, D for the output side, g = h/d):
     d_j  = v_j A(-s2_j) + q_j (Sp - B(-s2_j))
     h'_if = u_i C(-s1_i,f) + p_i (Sq_f - D(-s1_i,f))
  A,B,C,D are evaluated EXACTLY at G=128 grid thresholds via step-mask
  matmuls (O(N*G) work), then looked up per row by snapping -s2_j / -s1_i to
  the nearest grid point. The lookup itself is a matmul: by Abel summation,
  A(snap(t)) = sum_g [t >= th_g - D/2] * dA_g with dA = first difference of A
  (computed by a constant bidiagonal matmul). Errors near the kink are damped
  by |e^z - e^{0.2z}| ~ 0.8|z|, giving ~1e-4 grid error (bf16 noise ~4e-3
  dominates; tol is 2e-2). The Sq/Sp terms cancel exactly against the
  telescoping head (row 0 of the negated difference is zeroed).
  Total per-core work is O(N*(G+F)) - no N^2 pass anywhere.

Schedule notes: x is sent transposed and in bf16 (s1/s2 lose ~0.005 abs which
the error budget absorbs); xt/wsa DMAs go out first on all three DMA-capable
queues (sync/scalar/gpsimd) since per-queue bandwidth (~50GB/s) gates the h
matmuls; the s1/s2 rows for the broadcasts are accumulated into one [8,512]
PSUM tile via a zero-padded selector stationary (one copy instead of four
2-partition casts); junk-matmul bursts at the PE phase gaps keep the HAM
clock at 2.4GHz; the per-tile hs2 scaling is two broadcast tensor_tensor ops.
"""

import numpy as np
from contextlib import ExitStack

import concourse.bass as bass
import concourse.mybir as mybir
import concourse.tile as tile
from concourse import bacc
from concourse._compat import with_exitstack
from concourse.bass_utils import run_bass_kernel_spmd

F32 = mybir.dt.float32
F32R = mybir.dt.float32r
BF16 = mybir.dt.bfloat16
AF = mybir.ActivationFunctionType
ALU = mybir.AluOpType

N = 2048
FIN = 128
F = 64
P = 128
T = N // P            # 16 row tiles
G = 128               # grid size (incl. sentinel row 0)
LO, HI = -7.0, 7.0
DLT = (HI - LO) / (G - 2)
NEG_SLOPE = 0.2
N_CORES = 8


def _consts():
    from ml_dtypes import bfloat16
    th = np.concatenate([[-1.0e9], np.linspace(LO, HI, G - 1)]).astype(np.float32)
    thr = (DLT / 2 - th).astype(np.float32)          # T_d / T_h threshold
    ethr = np.exp(np.clip(thr, -80, 80)).astype(np.float32)
    ethr[0] = 3.0e38                                  # sentinel: always true
    thb = np.broadcast_to(th, (P, G)).astype(bfloat16)
    dmat = np.zeros((G, G), np.float32)
    for g in range(G):
        dmat[g, g] = 1.0
        if g > 0:
            dmat[g - 1, g] = -1.0
    # one bf16 const block: [thb | dmat | -dmat]
    cbf = np.concatenate([thb, dmat.astype(bfloat16), (-dmat).astype(bfloat16)],
                         axis=1)
    # one f32 const block: [thr | ethr]
    cf32 = np.concatenate([thr.reshape(G, 1), ethr.reshape(G, 1)], axis=1)
    sel8 = np.zeros((8, 8 * P), np.float32)
    for v in range(8):
        sel8[v, v * P:(v + 1) * P] = 1.0   # variant v broadcasts sr8 row v
    return {
        "cbf": np.ascontiguousarray(cbf),
        "cf32": np.ascontiguousarray(cf32),
        "sel8": np.ascontiguousarray(sel8.astype(bfloat16)),
    }


@with_exitstack
def _gat_body(ctx: ExitStack, tc: tile.TileContext, xt_d, wsa_d, cbf_d, cf32_d,
              sel8_d, zsel_d, out_d):
    nc = tc.nc
    const = ctx.enter_context(tc.tile_pool(name="const", bufs=1))

    # ---- SBUF tiles ----
    xt = const.tile([P, N], BF16)          # x^T (fin on partitions)
    wsa = const.tile([FIN, F + 2], BF16)   # [W | W@a1 | W@a2]
    cbf = const.tile([P, 3 * G], BF16)     # [thb | dmat | dmatn]
    cf32 = const.tile([G, 2], F32)         # [thr | ethr]
    sel8 = const.tile([8, 8 * P], BF16)    # row-select columns for broadcasts
    zsel = const.tile([FIN, 32], BF16)     # [wsa s-cols at rows 2c:2c+2] per chunk
    srow8 = const.tile([8, 512], BF16)     # s1/s2 rows, chunk c at rows 2c:2c+2
    s2b = const.tile([P, N], BF16)         # s2 broadcast (for Td compares)
    ub = const.tile([P, N], BF16)          # e^{s1} broadcast
    pb = const.tile([P, N], BF16)          # e^{.2 s1} broadcast
    Th = const.tile([P, N], BF16)          # [s1_i <= DLT/2 - th_g] (g=partition)
    Thu = const.tile([P, N], BF16)         # Th * u_i
    Thp = const.tile([P, N], BF16)         # Th * p_i
    Td = const.tile([P, N], BF16)          # [s2_j <= DLT/2 - th_g]
    hcols = const.tile([P, T, F + 2], F32) # [h | s1 | s2] per tile
    h_bf = const.tile([P, T, F], BF16)     # h in bf16 (batched cast)
    Gi = const.tile([P, T * G], BF16)      # [th_g < s1_i]
    Gj = const.tile([P, T * G], BF16)      # [th_g < s2_j]
    up_bf = const.tile([P, T, 2], BF16)    # (u_i, p_i) cols per tile
    vq = const.tile([P, T, 2], F32)        # (v_j, q_j) cols per tile
    dcol = const.tile([P, T, 1], F32)
    rd = const.tile([P, T, 1], F32)
    vd = const.tile([P, T, 1], F32)        # v/d
    qd = const.tile([P, T, 1], F32)        # q/d
    tmp1 = const.tile([P, T, 1], F32)
    tmp2 = const.tile([P, T, 1], F32)
    hs2 = const.tile([P, 2, T, F], BF16)   # [v*h/d ; q*h/d] per tile
    AB_sb = const.tile([G, 2], BF16)
    dAB_sb = const.tile([G, 2], BF16)
    CD_sb = const.tile([G, 2 * F], BF16)
    lhsTa = const.tile([G, F], BF16)       # dC
    lhsTb = const.tile([G, F], BF16)       # -dD (row 0 zeroed)
    o_sb = const.tile([F, N], F32)
    wup = const.tile([P, 256], BF16)
    dummy = const.tile([1, P], F32)

    thb = cbf[:, 0:G]
    dmat = cbf[:, G:2 * G]
    dmatn = cbf[:, 2 * G:3 * G]
    thrc = cf32[:, 0:1]
    ethrc = cf32[:, 1:2]

    # ---- DMAs: wsa + xt first on 3 queues (they gate the h matmuls) ----
    nc.gpsimd.dma_start(wsa[:], wsa_d)
    qs = [nc.sync, nc.scalar, nc.gpsimd, nc.sync]
    for c in range(4):
        sl = slice(c * 512, (c + 1) * 512)
        qs[c].dma_start(xt[:, sl], xt_d[:, sl])
    nc.scalar.dma_start(cbf[:], cbf_d)
    nc.gpsimd.dma_start(sel8[:], sel8_d)
    nc.gpsimd.dma_start(zsel[:], zsel_d)
    nc.sync.dma_start(cf32[:], cf32_d)

    with tc.tile_pool(name="ps_h", bufs=1, space="PSUM") as ps_h_pool, \
         tc.tile_pool(name="ps_m", bufs=1, space="PSUM") as ps_m_pool, \
         tc.tile_pool(name="ps_sr", bufs=1, space="PSUM") as ps_sr_pool, \
         tc.tile_pool(name="ps_b", bufs=2, space="PSUM") as ps_b_pool:
        # PE warmup burst (trips HAM 1.2->2.4GHz while DMAs land)
        nc.vector.memset(wup[:], 0.0)
        nc.vector.memset(dummy[:], 1.0)
        junk_n = [0]

        def junk(k):
            for _ in range(k):
                psw = ps_m_pool.tile([P, 256], F32, tag="m",
                                     name=f"wup{junk_n[0]}")
                junk_n[0] += 1
                nc.tensor.matmul(psw[:], lhsT=wup[:, 0:P], rhs=wup[:],
                                 start=True, stop=True)

        junk(20)
        # ACT table prefetch (exp) during the DMA window
        nc.scalar.activation(dummy[:], dummy[:], AF.Exp)

        # ---- per-chunk: h matmuls + score-row accumulation ----
        sr8_ps = ps_sr_pool.tile([8, 512], F32, tag="sr", name="sr8")
        for c in range(4):
            sl = slice(c * 512, (c + 1) * 512)
            for t in range(4 * c, 4 * c + 4):
                ph = ps_h_pool.tile([P, F + 2], F32, tag=f"h{t % 2}",
                                    name=f"ph{t}")
                nc.tensor.matmul(ph[:], lhsT=xt[:, t * P:(t + 1) * P],
                                 rhs=wsa[:], start=True, stop=True)
                if t % 2 == 0:
                    nc.scalar.copy(hcols[:, t, :], ph[:])
                else:
                    nc.vector.tensor_copy(hcols[:, t, :], ph[:])
                # grid step masks from the s1/s2 columns
                nc.vector.tensor_scalar(
                    Gi[:, t * G:(t + 1) * G], thb, hcols[:, t, F:F + 1],
                    None, ALU.is_lt)
                nc.vector.tensor_scalar(
                    Gj[:, t * G:(t + 1) * G], thb, hcols[:, t, F + 1:F + 2],
                    None, ALU.is_lt)
            nc.tensor.matmul(sr8_ps[:], lhsT=zsel[:, 8 * c:8 * c + 8],
                             rhs=xt[:, sl], start=(c == 0), stop=(c == 3))
        nc.vector.tensor_copy(srow8[:], sr8_ps[:])

        # ---- broadcasts (K=8 row-select matmuls) + Td / ub / pb ----
        for c in range(4):
            sl = slice(c * 512, (c + 1) * 512)
            psb = ps_b_pool.tile([P, 512], F32, tag="b2", name=f"ps2b{c}")
            nc.tensor.matmul(psb[:], lhsT=sel8[:, (2 * c + 1) * P:
                                               (2 * c + 2) * P],
                             rhs=srow8[:], start=True, stop=True)
            nc.vector.tensor_copy(s2b[:, sl], psb[:])
            nc.vector.tensor_scalar(Td[:, sl], s2b[:, sl], thrc, None,
                                    ALU.is_le)
            psb1 = ps_b_pool.tile([P, 512], F32, tag="b1", name=f"ps1b{c}")
            nc.tensor.matmul(psb1[:], lhsT=sel8[:, (2 * c) * P:
                                                (2 * c + 1) * P],
                             rhs=srow8[:], start=True, stop=True)
            nc.scalar.activation(ub[:, sl], psb1[:], AF.Exp)
            nc.scalar.activation(pb[:, sl], psb1[:], AF.Exp, scale=NEG_SLOPE)
        junk(6)

        # ---- T masks (chunked so they pipeline behind the ub/pb exps) ----
        for c in range(4):
            sl = slice(c * 512, (c + 1) * 512)
            nc.vector.tensor_scalar(Th[:, sl], ub[:, sl], ethrc, None, ALU.is_le)
            nc.vector.tensor_mul(Thu[:, sl], Th[:, sl], ub[:, sl])
            nc.vector.tensor_mul(Thp[:, sl], Th[:, sl], pb[:, sl])

        # ---- batched h cast + column exps ----
        nc.vector.tensor_copy(h_bf[:], hcols[:, :, 0:F])
        nc.scalar.activation(up_bf[:, :, 0:1], hcols[:, :, F:F + 1], AF.Exp)
        nc.scalar.activation(up_bf[:, :, 1:2], hcols[:, :, F:F + 1], AF.Exp,
                             scale=NEG_SLOPE)
        nc.scalar.activation(vq[:, :, 0:1], hcols[:, :, F + 1:F + 2], AF.Exp)
        nc.scalar.activation(vq[:, :, 1:2], hcols[:, :, F + 1:F + 2], AF.Exp,
                             scale=NEG_SLOPE)

    # ---- phase 2: threshold sums on the grid, lookups, output ----
    with tc.tile_pool(name="ps_ab", bufs=1, space="PSUM") as ps_ab, \
         tc.tile_pool(name="ps_d", bufs=1, space="PSUM") as ps_d, \
         tc.tile_pool(name="ps_j2", bufs=1, space="PSUM") as ps_j2:

        def junk2(k, tag):
            for i in range(k):
                psw = ps_j2.tile([P, 256], F32, tag=tag, name=f"w2{tag}{i}")
                nc.tensor.matmul(psw[:], lhsT=wup[:, 0:P], rhs=wup[:],
                                 start=True, stop=True)

        AB_ps = ps_ab.tile([G, 2], F32, tag="ab")
        for t in range(T):
            nc.tensor.matmul(AB_ps[:], lhsT=Gi[:, t * G:(t + 1) * G],
                             rhs=up_bf[:, t, :],
                             start=(t == 0), stop=(t == T - 1))
        nc.vector.tensor_copy(AB_sb[:], AB_ps[:])
        dAB_ps = ps_ab.tile([G, 2], F32, tag="dab")
        nc.tensor.matmul(dAB_ps[:, 0:1], lhsT=dmat, rhs=AB_sb[:, 0:1],
                         start=True, stop=True)
        nc.tensor.matmul(dAB_ps[:, 1:2], lhsT=dmatn, rhs=AB_sb[:, 1:2],
                         start=True, stop=True)
        nc.vector.tensor_copy(dAB_sb[:], dAB_ps[:])
        nc.vector.memset(dAB_sb[0:1, 1:2], 0.0)   # (-dB)_0 = 0: Sp cancels

        # d lookup: dps[:, t, :] = [A, Sp-B] per node column block
        dps = ps_d.tile([P, T, 2], F32, tag="d")
        for t in range(T):
            nc.tensor.matmul(dps[:, t, :], lhsT=Td[:, t * P:(t + 1) * P],
                             rhs=dAB_sb[:], start=True, stop=True)
        junk2(16, "a")

        # d = v*A + q*(Sp-B); rd = 1/d; vd = v/d; qd = q/d
        nc.vector.tensor_mul(tmp1[:], vq[:, :, 0:1], dps[:, :, 0:1])
        nc.vector.tensor_mul(tmp2[:], vq[:, :, 1:2], dps[:, :, 1:2])
        nc.vector.tensor_add(dcol[:], tmp1[:], tmp2[:])
        nc.vector.reciprocal(rd[:], dcol[:])
        nc.vector.tensor_mul(vd[:], vq[:, :, 0:1], rd[:])
        nc.vector.tensor_mul(qd[:], vq[:, :, 1:2], rd[:])

        # hs2 = [h*vd ; h*qd] in bf16 - two broadcast tensor_tensor ops
        nc.vector.tensor_mul(hs2[:, 0, :, :], h_bf[:],
                             vd[:].broadcast_to([P, T, F]))
        nc.vector.tensor_mul(hs2[:, 1, :, :], h_bf[:],
                             qd[:].broadcast_to([P, T, F]))

    with tc.tile_pool(name="ps_cd", bufs=1, space="PSUM") as ps_cd, \
         tc.tile_pool(name="ps_o", bufs=1, space="PSUM") as ps_o, \
         tc.tile_pool(name="ps_j3", bufs=1, space="PSUM") as ps_j3:
        def junk3(k, tag):
            for i in range(k):
                psw = ps_j3.tile([P, 256], F32, tag=tag, name=f"w3{tag}{i}")
                nc.tensor.matmul(psw[:], lhsT=wup[:, 0:P], rhs=wup[:],
                                 start=True, stop=True)

        junk3(10, "a")
        CD_ps = ps_cd.tile([G, 2 * F], F32, tag="cd")
        for t in range(T):
            nc.tensor.matmul(CD_ps[:], lhsT=Gj[:, t * G:(t + 1) * G],
                             rhs=hs2[:, :, t, :],
                             start=(t == 0), stop=(t == T - 1))
        nc.vector.tensor_copy(CD_sb[:], CD_ps[:])
        dCD_ps = ps_cd.tile([G, 2 * F], F32, tag="dcd")
        nc.tensor.matmul(dCD_ps[:, 0:F], lhsT=dmat, rhs=CD_sb[:, 0:F],
                         start=True, stop=True)
        nc.tensor.matmul(dCD_ps[:, F:2 * F], lhsT=dmatn, rhs=CD_sb[:, F:2 * F],
                         start=True, stop=True)
        nc.vector.tensor_copy(lhsTa[:], dCD_ps[:, 0:F])
        nc.vector.tensor_copy(lhsTb[:], dCD_ps[:, F:2 * F])
        nc.vector.memset(lhsTb[0:1, :], 0.0)   # (-dD)_0 = 0: Sq cancels
        junk3(8, "b")

        # h'^T = dC^T @ Thu + (-dD)^T @ Thp, then lrelu + DMA out
        hp_ps = [ps_o.tile([F, 512], F32, tag=f"o{c}", name=f"hp{c}")
                 for c in range(4)]
        for c in range(4):
            nc.tensor.matmul(hp_ps[c][:], lhsT=lhsTa[:],
                             rhs=Thu[:, c * 512:(c + 1) * 512],
                             start=True, stop=False)
        for c in range(4):
            nc.tensor.matmul(hp_ps[c][:], lhsT=lhsTb[:],
                             rhs=Thp[:, c * 512:(c + 1) * 512],
                             start=False, stop=True)
            sl = slice(c * 512, (c + 1) * 512)
            nc.scalar.activation(o_sb[:, sl], hp_ps[c][:], AF.Prelu,
                                 bias=0.0, scale=1.0, alpha=NEG_SLOPE)
            eng = (nc.sync, nc.gpsimd)[c % 2]
            eng.dma_start(out_d[:, sl], o_sb[:, sl])


_NC_CACHE = {}


def _build_nc():
    if "nc" in _NC_CACHE:
        return _NC_CACHE["nc"]
    nc = bacc.Bacc("TRN2", target_bir_lowering=False, debug=False)
    xt = nc.dram_tensor("xt", (P, N), BF16, kind="ExternalInput").ap()
    wsa = nc.dram_tensor("wsa", (FIN, F + 2), BF16, kind="ExternalInput").ap()
    cbf = nc.dram_tensor("cbf", (P, 3 * G), BF16, kind="ExternalInput").ap()
    cf32 = nc.dram_tensor("cf32", (G, 2), F32, kind="ExternalInput").ap()
    sel8 = nc.dram_tensor("sel8", (8, 8 * P), BF16, kind="ExternalInput").ap()
    zsel = nc.dram_tensor("zsel", (FIN, 32), BF16, kind="ExternalInput").ap()
    out = nc.dram_tensor("out", (F, N), F32, kind="ExternalOutput").ap()
    with tile.TileContext(nc) as tc:
        _gat_body(tc, xt, wsa, cbf, cf32, sel8, zsel, out)
    nc.compile()
    _NC_CACHE["nc"] = nc
    return nc


def make_in_maps(x, W, a):
    from ml_dtypes import bfloat16
    x = np.ascontiguousarray(np.asarray(x), dtype=np.float32)
    W = np.ascontiguousarray(np.asarray(W), dtype=np.float32)
    a = np.ascontiguousarray(np.asarray(a), dtype=np.float32)
    assert x.shape == (N_CORES, N, FIN), x.shape
    wsa = np.concatenate([W, W @ a[:F], W @ a[F:]], axis=1).astype(np.float32)
    zsel = np.zeros((FIN, 32), np.float32)
    for c in range(4):
        zsel[:, 8 * c + 2 * c:8 * c + 2 * c + 2] = wsa[:, F:F + 2]
    shared = {"wsa": np.ascontiguousarray(wsa.astype(bfloat16)),
              "zsel": np.ascontiguousarray(zsel.astype(bfloat16)), **_consts()}
    return [{"xt": np.ascontiguousarray(x[c].T.astype(bfloat16)), **shared}
            for c in range(N_CORES)]


def kernel(x, W, a):
    nc = _build_nc()
    in_maps = make_in_maps(x, W, a)
    res = run_bass_kernel_spmd(nc, in_maps, core_ids=list(range(N_CORES)))
    return np.stack([res.results[c]["out"].T.copy() for c in range(N_CORES)],
                    axis=0)
